# revision 10
# baseline (speedup 1.0000x reference)
"""Trainium2 Bass kernel for ToRA-adapted windowed attention block.

Math (per batch image, S=1024 tokens, dim=768, 12 heads x 64):
  qkv  = x @ (Wqkv + U1 Gt U2^T)^T + b          Gt = G . U3[task]
  q,k,v split; attn = softmax(q k^T / 8) v ; out = attn-merge
  y    = out @ (Wp + U1p Gtp U2p^T)^T + bp

Strategy: data-parallel over B=8 -- one image per NeuronCore, no
collectives. Device pipeline (v2):
  - host precomputes effective weights; DMA ordered x -> Wqk (by ft
    column chunk) -> Wv so the QK matmuls start ~10us in and all
    later loads hide under compute.
  - QK^T computed feature-major; V computed token-major into a
    bf16 Vaug tile (ones column per head) via one strided DVE add.
  - per-head stats pass (row-max or ACT-accum LSE bound) overlapped
    under the QKV stage; result staged in a persistent crow tile.
  - phase2 scores k-major with the [K;1]/[Q;-c] ones-row fold, exp in
    1024-wide ACT ops into bf16 A^T tiles.
  - AV runs "V-moving": stationary A^T tile, moving 65-wide V+ones =>
    token-major out with denominator per partition; normalize is a
    per-partition DVE multiply, then PE-transpose into feature-major
    attnT for proj. proj is interleaved with the last head's AV.
All big matmuls run as float32r / bf16 (1 cycle/row).
"""

import os
import sys
from collections import deque

import numpy as np

sys.path.insert(0, "/opt/trn_rl_repo")

import concourse.bass as bass
import concourse.tile as tile
from concourse import bacc, mybir
from concourse.bass_utils import run_bass_kernel_spmd
from concourse.masks import make_identity

F32 = mybir.dt.float32
F32R = mybir.dt.float32r
BF16 = mybir.dt.bfloat16
I32 = mybir.dt.int32
AX = mybir.AxisListType.X
OP = mybir.AluOpType
EXP = mybir.ActivationFunctionType.Exp
IDENT = mybir.ActivationFunctionType.Identity

D = 768          # model dim
KT = 6           # contract tiles over D
S = 1024         # tokens per image
NH = 12
HD = 64
MARGIN = 1.0     # safety margin over the stats-pass row-max
LN2 = 0.6931471805599453

N_CORES = 8

FTORD = [0, 6, 1, 7, 2, 8, 3, 9, 4, 10, 5, 11]


def build_program():
    SKIP = set(os.environ.get("K_SKIP", "").split(","))
    LSE_HEADS = {
        int(v) for v in os.environ.get("K_LSE", "0,1,2,3").split(",") if v != ""
    }
    nc = bacc.Bacc(
        "TRN2",
        target_bir_lowering=False,
        debug=False,
        enable_asserts=True,
        num_devices=N_CORES,
    )
    xT = nc.dram_tensor("xT", [D, S], F32, kind="ExternalInput").ap()
    WqkT = nc.dram_tensor("WqkT", [D, 2 * D], F32, kind="ExternalInput").ap()
    WvT = nc.dram_tensor("WvT", [D, D], F32, kind="ExternalInput").ap()
    WpT = nc.dram_tensor("WpT", [D, D], F32, kind="ExternalInput").ap()
    bqk = nc.dram_tensor("bqk", [128, 12], F32, kind="ExternalInput").ap()
    bv = nc.dram_tensor("bv", [D], F32, kind="ExternalInput").ap()
    bp = nc.dram_tensor("bp", [D], F32, kind="ExternalInput").ap()
    y = nc.dram_tensor("y", [S, D], F32, kind="ExternalOutput").ap()

    def bcast128(v):
        return bass.AP(tensor=v.tensor, offset=v.offset, ap=[[0, 128], [1, D]])

    with tile.TileContext(nc) as tc:
        with (
            tc.tile_pool(name="persist", bufs=1) as persist,
            tc.tile_pool(name="statp", bufs=2) as statp,
            tc.tile_pool(name="statps", bufs=1, space="PSUM") as statps,
            tc.tile_pool(name="smallps", bufs=2, space="PSUM") as smallps,
        ):
            QKT = persist.tile([128, 12, S], F32R)
            Vaug = persist.tile([128, 8, NH, 65], BF16)
            attnT = persist.tile([128, KT, S], F32R)
            bqk_sb = persist.tile([128, 12], F32)
            bvb = persist.tile([128, NH, HD], F32)
            bpb = persist.tile([128, D], F32)
            identf = persist.tile([128, 128], F32)
            identr = persist.tile([128, 128], F32R)
            crow = persist.tile([8, NH, 128], F32)

            nc.sync.dma_start(out=bqk_sb, in_=bqk)
            nc.sync.dma_start(out=bvb, in_=bcast128(bv))
            nc.sync.dma_start(out=bpb, in_=bcast128(bp))
            make_identity(nc, identf)
            nc.vector.tensor_scalar(
                out=identr, in0=identf, scalar1=1.0, scalar2=None, op0=OP.mult
            )
            nc.vector.memset(Vaug, 1.0)

            # ---------------- phase1: per-head row-stat pass ----------------
            def phase1(h):
                fq, off = h // 2, (h % 2) * 64
                fk = 6 + fq
                use_lse = h in LSE_HEADS
                mcol = statp.tile([128, 8], F32, tag="mcol")
                for qt in range(8):
                    sp = statps.tile([128, 1024], F32, tag="st")
                    for kc in range(2):
                        nc.tensor.matmul(
                            sp[:, kc * 512 : (kc + 1) * 512],
                            QKT[off : off + 64, fq, qt * 128 : (qt + 1) * 128],
                            QKT[off : off + 64, fk, kc * 512 : (kc + 1) * 512],
                            start=True,
                            stop=True,
                        )
                    if use_lse:
                        esc = statp.tile([128, 1024], F32, tag="esc")
                        nc.scalar.activation(
                            esc, sp, EXP, scale=0.0625,
                            accum_out=mcol[:, qt : qt + 1],
                        )
                    else:
                        nc.vector.reduce_max(mcol[:, qt : qt + 1], sp, axis=AX)
                    yield
                # row-ify -c into crow[:, h, :] via PE transpose
                psT = smallps.tile([128, 128], F32R if use_lse else F32,
                                   tag="sm", name="psT")
                if use_lse:
                    # bit-hack log2: c = 16*ln2*(bits/2^23 - 127) - 40
                    mbits = statp.tile([128, 8], F32R, tag="mbits")
                    nc.vector.tensor_copy(mbits, mcol.bitcast(I32))
                    nc.tensor.transpose(psT[0:8, :], mbits, identr)
                    nc.vector.tensor_scalar(
                        out=crow[:, h, :], in0=psT[0:8, :],
                        scalar1=-16.0 * LN2 / 8388608.0,
                        scalar2=16.0 * 127.0 * LN2 + 40.0,
                        op0=OP.mult, op1=OP.add,
                    )
                else:
                    nc.tensor.transpose(psT[0:8, :], mcol, identf)
                    nc.vector.tensor_scalar(
                        out=crow[:, h, :], in0=psT[0:8, :],
                        scalar1=-1.0, scalar2=-MARGIN,
                        op0=OP.mult, op1=OP.add,
                    )
                yield

            # ---------------- stage A: QKV + phase1 overlap ----------------
            with (
                tc.tile_pool(name="stageA", bufs=1) as A,
                tc.tile_pool(name="qkps", bufs=2, space="PSUM") as qkps,
                tc.tile_pool(name="vps", bufs=1, space="PSUM") as vps,
            ):
                xT_sb = A.tile([128, KT, S], F32R)
                WqkT_sb = A.tile([128, KT, 2 * D], F32R)
                WvT_sb = A.tile([128, KT, D], F32R)
                xT_r = xT.rearrange("(k p) t -> p k t", p=128).bitcast(F32R)
                WqkT_r = WqkT.rearrange("(k p) f -> p k f", p=128).bitcast(F32R)
                WvT_r = WvT.rearrange("(k p) f -> p k f", p=128).bitcast(F32R)
                for kt in range(KT):
                    nc.sync.dma_start(out=xT_sb[:, kt, :], in_=xT_r[:, kt, :])
                for ft in FTORD:
                    nc.sync.dma_start(
                        out=WqkT_sb[:, :, ft * 128 : (ft + 1) * 128],
                        in_=WqkT_r[:, :, ft * 128 : (ft + 1) * 128],
                    )
                for kt in range(KT):
                    nc.sync.dma_start(out=WvT_sb[:, kt, :], in_=WvT_r[:, kt, :])

                def qk_ft(ft):
                    ps_a = qkps.tile([128, 512], F32, tag="qk", name="ps_a")
                    ps_b = qkps.tile([128, 512], F32, tag="qk", name="ps_b")
                    pss2 = [ps_a, ps_b]
                    for kt in range(KT):
                        for qc in range(2):
                            nc.tensor.matmul(
                                pss2[qc],
                                WqkT_sb[:, kt, ft * 128 : (ft + 1) * 128],
                                xT_sb[:, kt, qc * 512 : (qc + 1) * 512],
                                start=(kt == 0),
                                stop=(kt == KT - 1),
                            )
                    for qc in range(2):
                        # DVE evac: psum*scale + bias(ptr); host pre-scales
                        # the Q bias by 1/8 so (raw+b)/8 == raw*0.125 + b/8
                        nc.vector.tensor_scalar(
                            out=QKT[:, ft, qc * 512 : (qc + 1) * 512],
                            in0=pss2[qc],
                            scalar1=0.125 if ft < 6 else 1.0,
                            scalar2=bqk_sb[:, ft : ft + 1],
                            op0=OP.mult, op1=OP.add,
                        )

                def v_tt(tt):
                    psv = vps.tile([128, NH, HD], F32, tag="v")
                    for kt in range(KT):
                        nc.tensor.matmul(
                            psv[:, 0:8, :],
                            xT_sb[:, kt, tt * 128 : (tt + 1) * 128],
                            WvT_sb[:, kt, 0:512],
                            start=(kt == 0),
                            stop=(kt == KT - 1),
                        )
                        nc.tensor.matmul(
                            psv[:, 8:12, :],
                            xT_sb[:, kt, tt * 128 : (tt + 1) * 128],
                            WvT_sb[:, kt, 512:768],
                            start=(kt == 0),
                            stop=(kt == KT - 1),
                        )
                    nc.vector.tensor_tensor(
                        Vaug[:, tt, :, 0:64], psv, bvb, OP.add
                    )

                ph1 = deque()          # backlog of (head, generator)
                ph1_active = deque()   # <= 2 in flight (mcol/esc have bufs=2;
                                       # more interleaved heads deadlocks slot
                                       # WAR against the in-order PE queue)
                ph1_done = set()

                def pump(n):
                    for _ in range(n):
                        while len(ph1_active) < 2 and ph1:
                            ph1_active.append(ph1.popleft())
                        if not ph1_active:
                            return
                        h, g = ph1_active.popleft()
                        try:
                            next(g)
                            ph1_active.append((h, g))
                        except StopIteration:
                            ph1_done.add(h)

                def ensure_ph1_done(h):
                    # drain stats generators (FIFO) until head h's is complete,
                    # so phase2(h)'s crow read is emitted after phase1(h)'s write
                    while (ph1 or ph1_active) and h not in ph1_done:
                        pump(1)

                if "qkv" not in SKIP:
                    heads_p1 = [] if "attn" in SKIP else list(range(NH))
                    for j, ft in enumerate(FTORD):
                        qk_ft(ft)
                        # heads of pair j-1 become available after pair j-1's
                        # evacs; emit their stats interleaved one pair late
                        if j % 2 == 1 and j >= 3:
                            pair = (j - 3) // 2
                            for h in (2 * pair, 2 * pair + 1):
                                if h in heads_p1:
                                    ph1.append((h, phase1(h)))
                        pump(2)
                    for h in (8, 9, 10, 11):
                        if h in heads_p1:
                            ph1.append((h, phase1(h)))
                    for tt in range(8):
                        v_tt(tt)
                        pump(3)

            # ---------------- stage B: attention + proj ----------------
            with (
                tc.tile_pool(name="stageB", bufs=1) as B,
                tc.tile_pool(name="qkbuf", bufs=4) as qkbuf,
                tc.tile_pool(name="atp", bufs=2) as atp,
                tc.tile_pool(name="ysb", bufs=2) as ysb,
                tc.tile_pool(name="scps", bufs=2, space="PSUM") as scps,
            ):
                WpT_sb = B.tile([128, KT, D], F32R)
                WpT_r = WpT.rearrange("(k p) f -> p k f", p=128).bitcast(F32R)
                for kt in range(KT):
                    nc.sync.dma_start(out=WpT_sb[:, kt, :], in_=WpT_r[:, kt, :])

                def phase2(hp):
                    # head pair (2hp, 2hp+1): shares ft tiles, one combined
                    # [128,128] transpose + evac per qt
                    he, ho = 2 * hp, 2 * hp + 1
                    ensure_ph1_done(he)
                    ensure_ph1_done(ho)
                    fq, fk = hp, 6 + hp
                    Kts, Qts, ATs = {}, {}, {}
                    for h, off in ((he, 0), (ho, 64)):
                        Kt = qkbuf.tile([65, S], F32R, tag="Kt")
                        Qt = qkbuf.tile([65, S], F32R, tag="Qt")
                        nc.gpsimd.tensor_copy(
                            Kt[0:64, :], QKT[off : off + 64, fk, :]
                        )
                        nc.gpsimd.memset(
                            Kt[64:65, :].bitcast(mybir.dt.uint32), 0x3F800000
                        )
                        nc.gpsimd.tensor_copy(
                            Qt[0:64, :], QKT[off : off + 64, fq, :]
                        )
                        nc.sync.dma_start(
                            out=Qt[64:65, :].bitcast(F32), in_=crow[:, h, :]
                        )
                        Kts[h], Qts[h] = Kt, Qt
                        yield
                    for h in (he, ho):
                        AT = atp.tile([128, 8, S], BF16, tag="AT")
                        for kt in range(8):
                            sc = scps.tile([128, 1024], F32, tag="sc")
                            for qc in range(2):
                                nc.tensor.matmul(
                                    sc[:, qc * 512 : (qc + 1) * 512],
                                    Kts[h][:, kt * 128 : (kt + 1) * 128],
                                    Qts[h][:, qc * 512 : (qc + 1) * 512],
                                    start=True,
                                    stop=True,
                                )
                            nc.scalar.activation(AT[:, kt, :], sc, EXP)
                            yield
                        ATs[h] = AT
                    for qt in range(8):
                        nsb2 = statp.tile([128, 128], F32R, tag="nsb")
                        for i, h in enumerate((he, ho)):
                            pso = smallps.tile([128, 128], F32, tag="sm",
                                               name="pso")
                            for kt in range(8):
                                nc.tensor.matmul(
                                    pso[:, 0:65],
                                    ATs[h][:, kt, qt * 128 : (qt + 1) * 128],
                                    Vaug[:, kt, h, :],
                                    start=(kt == 0),
                                    stop=(kt == 7),
                                )
                            rec = statp.tile([128, 1], F32, tag="rec")
                            nc.vector.reciprocal(rec, pso[:, 64:65])
                            # softmax 1/l is a per-partition (per-token) scale
                            nc.scalar.mul(
                                nsb2[:, i * 64 : (i + 1) * 64],
                                pso[:, 0:64], rec,
                            )
                        psT2 = smallps.tile([128, 128], F32R, tag="sm",
                                            name="psT2")
                        nc.tensor.transpose(psT2, nsb2, identr)
                        nc.vector.tensor_copy(
                            attnT[:, hp, qt * 128 : (qt + 1) * 128], psT2
                        )
                        yield ("av", hp, qt)

                def proj_tt(tt):
                    psy = scps.tile([128, 1024], F32, tag="sc", name="psy")
                    for f0, fl in ((0, 512), (512, 256)):
                        for kt in range(KT):
                            nc.tensor.matmul(
                                psy[:, f0 : f0 + fl],
                                attnT[:, kt, tt * 128 : (tt + 1) * 128],
                                WpT_sb[:, kt, f0 : f0 + fl],
                                start=(kt == 0),
                                stop=(kt == KT - 1),
                            )
                    yt = ysb.tile([128, D], F32, tag="yt")
                    nc.vector.tensor_tensor(yt, psy[:, 0:768], bpb, OP.add)
                    nc.sync.dma_start(out=y[tt * 128 : (tt + 1) * 128, :], in_=yt)

                pairs = [] if "attn" in SKIP else list(range(NH // 2))
                do_proj = "proj" not in SKIP
                last_p = pairs[-1] if pairs else None
                gens = [phase2(p) for p in pairs]
                started = [False] * len(gens)

                def prefetch(i):
                    # advance pair i through its 2 Kt/Qt-copy yields
                    if i < len(gens) and not started[i]:
                        next(gens[i])
                        next(gens[i])
                        started[i] = True

                for i, p in enumerate(pairs):
                    prefetch(i)
                    prefetch(i + 1)
                    g = gens[i]
                    while True:
                        if ph1 or ph1_active:
                            pump(1)
                        try:
                            r = next(g)
                        except StopIteration:
                            break
                        if r is not None and r[0] == "av" and r[1] == last_p \
                                and do_proj:
                            proj_tt(r[2])
                while ph1 or ph1_active:
                    pump(1)
                if do_proj and last_p is None:
                    for tt in range(8):
                        proj_tt(tt)

    nc.compile()
    return nc


_NC = None


def _get_nc():
    global _NC
    if _NC is None:
        _NC = build_program()
    return _NC


def prep_inputs(x, qkv_w, qkv_b, U1_qkv, U2_qkv, U3_qkv, G_qkv,
                proj_w, proj_b, U1_p, U2_p, U3_p, G_p, task_idx):
    t = int(task_idx)
    f = np.float32
    x = np.asarray(x, f)
    qkv_w = np.asarray(qkv_w, f)
    qkv_b = np.asarray(qkv_b, f)
    proj_w = np.asarray(proj_w, f)
    proj_b = np.asarray(proj_b, f)

    Gt = np.einsum("pqv,v->pq", np.asarray(G_qkv, f), np.asarray(U3_qkv, f)[t])
    Wqkv = qkv_w + np.asarray(U1_qkv, f) @ Gt @ np.asarray(U2_qkv, f).T
    Gtp = np.einsum("pqv,v->pq", np.asarray(G_p, f), np.asarray(U3_p, f)[t])
    Wp = proj_w + np.asarray(U1_p, f) @ Gtp @ np.asarray(U2_p, f).T

    WqkT = np.ascontiguousarray(Wqkv[: 2 * D].T)
    WvT = np.ascontiguousarray(Wqkv[2 * D :].T)
    WpT = np.ascontiguousarray(Wp.T)
    bqk = np.ascontiguousarray(qkv_b[: 2 * D].reshape(12, 128).T)
    bqk[:, 0:6] *= 0.125  # Q bias pre-scaled (ACT evac applies scale to psum only)
    bv = np.ascontiguousarray(qkv_b[2 * D :])
    bp = proj_b

    B = x.shape[0]
    xr = x.reshape(B, S, D)
    in_maps = [
        dict(
            xT=np.ascontiguousarray(xr[c].T),
            WqkT=WqkT, WvT=WvT, WpT=WpT, bqk=bqk, bv=bv, bp=bp,
        )
        for c in range(B)
    ]
    return in_maps


def run(in_maps, trace=False):
    nc = _get_nc()
    res = run_bass_kernel_spmd(nc, in_maps, list(range(N_CORES)), trace=trace)
    return res


def kernel(x, **kw):
    B, H, W, C = x.shape
    in_maps = prep_inputs(x, **kw)
    res = run(in_maps)
    out = np.stack([np.asarray(res.results[c]["y"]) for c in range(B)])
    return out.reshape(B, H, W, C).astype(np.float32)


# revision 21
# speedup vs baseline: 1.2826x; 1.2826x over previous
"""Trainium2 Bass kernel for ToRA-adapted windowed attention block.

Math (per batch image, S=1024 tokens, dim=768, 12 heads x 64):
  qkv  = x @ (Wqkv + U1 Gt U2^T)^T + b          Gt = G . U3[task]
  q,k,v split; attn = softmax(q k^T / 8) v ; out = attn-merge
  y    = out @ (Wp + U1p Gtp U2p^T)^T + bp

Strategy: data-parallel over B=8 — one image per NeuronCore, no
collectives. Device pipeline is feature-major:
  - host pre-computes effective weights (tiny low-rank update) and
    pre-transposes weights + x so the contract dim lands on SBUF
    partitions.
  - QKV^T computed feature-major for Q,K (gives Q^T/K^T tiles directly);
    V computed token-major and augmented with a ones-column so the
    A^T @ V_aug matmul also yields softmax denominators for free.
  - scores are computed twice on PE (cheap with f32r @ 1 cyc/row):
    once q-major subsampled (stride 4) for row-max stats, once k-major
    with an appended (-max - margin) row folded into the contraction, so
    exp() needs no per-column bias and A^T comes out k-major, ready to
    contract with V.
  - proj consumes attention output feature-major; softmax 1/l scaling is
    applied during the attention evacuation via a PE rank-1 broadcast.
All big matmuls run as float32r (FP22 multiply, FP32 accumulate).
"""

import os
import sys
import numpy as np

sys.path.insert(0, "/opt/trn_rl_repo")

import concourse.bass as bass
import concourse.tile as tile
from concourse import bacc, mybir
from concourse.bass_utils import run_bass_kernel_spmd
from concourse.masks import make_identity

F32 = mybir.dt.float32
F32R = mybir.dt.float32r
BF16 = mybir.dt.bfloat16
AX = mybir.AxisListType.X
OP = mybir.AluOpType
EXP = mybir.ActivationFunctionType.Exp
LN = mybir.ActivationFunctionType.Ln
IDENT = mybir.ActivationFunctionType.Identity

D = 768          # model dim
KT = 6           # contract tiles over D
S = 1024         # tokens per image
NH = 12
HD = 64
MARGIN = 1.0     # safety margin over the stats-pass row-max

N_CORES = 8


def build_program():
    import os as _os
    SKIP = set(_os.environ.get("K_SKIP", "").split(","))
    LSE_HEADS = {
        int(v) for v in _os.environ.get("K_LSE", "2,6,10").split(",") if v != ""
    }
    nc = bacc.Bacc(
        "TRN2",
        target_bir_lowering=False,
        debug=False,
        enable_asserts=True,
        num_devices=N_CORES,
    )
    xT = nc.dram_tensor("xT", [D, S], F32, kind="ExternalInput").ap()
    WqkT = nc.dram_tensor("WqkT", [D, 2 * D], F32, kind="ExternalInput").ap()
    WvT = nc.dram_tensor("WvT", [D, D], F32, kind="ExternalInput").ap()
    WpT = nc.dram_tensor("WpT", [D, D], F32, kind="ExternalInput").ap()
    bqk = nc.dram_tensor("bqk", [128, 12], F32, kind="ExternalInput").ap()
    bv = nc.dram_tensor("bv", [D], F32, kind="ExternalInput").ap()
    bp = nc.dram_tensor("bp", [D], F32, kind="ExternalInput").ap()
    y = nc.dram_tensor("y", [S, D], F32, kind="ExternalOutput").ap()

    def bcast128(v):
        return bass.AP(tensor=v.tensor, offset=v.offset, ap=[[0, 128], [1, D]])

    with tile.TileContext(nc) as tc:
        with tc.tile_pool(name="persist", bufs=1) as persist:
            # long-lived tiles
            QKT = persist.tile([128, 12, S], F32R)      # Q^T, K^T feature-major
            Vaug = persist.tile([128, 8, NH, 65], BF16)  # V tok-major + ones col
            attnT = persist.tile([128, KT, S], F32R)      # attn out^T, normalized
            bqk_sb = persist.tile([128, 12], F32)
            bvb3 = persist.tile([128, NH, HD], F32)
            bpb = persist.tile([128, D], F32)
            ident = persist.tile([128, 128], F32)
            identr = persist.tile([128, 128], F32R)
            ones1 = persist.tile([1, 64], F32R)

            nc.sync.dma_start(out=bqk_sb, in_=bqk)
            nc.sync.dma_start(out=bvb3, in_=bcast128(bv))
            nc.sync.dma_start(out=bpb, in_=bcast128(bp))
            make_identity(nc, ident)
            nc.vector.tensor_scalar(
                out=identr, in0=ident, scalar1=1.0, scalar2=None, op0=OP.mult
            )
            ONE_F32 = 0x3F800000
            nc.vector.memset(ones1.bitcast(mybir.dt.uint32), ONE_F32)
            nc.vector.memset(Vaug, 1.0)

            # ---------------- stage A: QKV ----------------
            with (
                tc.tile_pool(name="qkvw", bufs=1) as qkvw,
                tc.tile_pool(name="qkps", bufs=3, space="PSUM") as qkps,
                tc.tile_pool(name="vps", bufs=2, space="PSUM") as vps,
            ):
                xT_sb = qkvw.tile([128, KT, S], F32R)
                WqkT_sb = qkvw.tile([128, KT, 2 * D], F32R)
                WvT_sb = qkvw.tile([128, KT, D], F32R)
                xT_r = xT.rearrange("(k p) t -> p k t", p=128).bitcast(F32R)
                WqkT_r = WqkT.rearrange("(k p) f -> p k f", p=128).bitcast(F32R)
                WvT_r = WvT.rearrange("(k p) f -> p k f", p=128).bitcast(F32R)

                def dma_wqk(ft, klo, khi):
                    nc.sync.dma_start(
                        out=WqkT_sb[:, klo:khi, ft * 128 : (ft + 1) * 128],
                        in_=WqkT_r[:, klo:khi, ft * 128 : (ft + 1) * 128],
                    )

                FTORD = [0, 6, 1, 7, 2, 8, 3, 9, 4, 10, 5, 11]
                # first ft pair interleaved with x so QK matmuls start ~2us in
                dma_wqk(0, 0, 3)
                nc.sync.dma_start(out=xT_sb[:, 0, :], in_=xT_r[:, 0, :])
                nc.sync.dma_start(out=xT_sb[:, 1, :], in_=xT_r[:, 1, :])
                dma_wqk(0, 3, 6)
                dma_wqk(6, 0, 6)
                for kt in range(2, KT):
                    nc.sync.dma_start(out=xT_sb[:, kt, :], in_=xT_r[:, kt, :])
                for ft in FTORD[2:]:
                    dma_wqk(ft, 0, 6)
                for kt in range(KT):
                    nc.sync.dma_start(out=WvT_sb[:, kt, :], in_=WvT_r[:, kt, :])

                # V token-major: one strided add into bf16 Vaug (+bias)
                def v_tt(tt):
                    psv = vps.tile([128, NH, HD], F32, tag="psv")
                    for kt in range(KT):
                        nc.tensor.matmul(
                            psv[:, 0:8, :],
                            (xT_sb[:, kt, tt * 128 : (tt + 1) * 128]),
                            (WvT_sb[:, kt, 0:512]),
                            start=(kt == 0),
                            stop=(kt == KT - 1),
                        )
                        nc.tensor.matmul(
                            psv[:, 8:12, :],
                            (xT_sb[:, kt, tt * 128 : (tt + 1) * 128]),
                            (WvT_sb[:, kt, 512:768]),
                            start=(kt == 0),
                            stop=(kt == KT - 1),
                        )
                    nc.vector.tensor_tensor(
                        Vaug[:, tt, :, 0:64], psv, bvb3, OP.add
                    )

                # Q^T / K^T feature-major: out[feat, tok]
                for ft in ([] if 'qkv' in SKIP else [0, 6, 1, 7, 2, 8, 3, 9, 4, 10, 5, 11]):
                    ps_a = qkps.tile([128, 512], F32, tag="qkps")
                    ps_b = qkps.tile([128, 512], F32, tag="qkps")
                    pss2 = [ps_a, ps_b]
                    for kt in range(KT):
                        for qc in range(2):
                            nc.tensor.matmul(
                                pss2[qc],
                                (WqkT_sb[:, kt, ft * 128 : (ft + 1) * 128]),
                                (xT_sb[:, kt, qc * 512 : (qc + 1) * 512]),
                                start=(kt == 0),
                                stop=(kt == KT - 1),
                            )
                    for qc in range(2):
                        dst = QKT[:, ft, qc * 512 : (qc + 1) * 512]
                        # ACT evac: Identity(psum*scale + bias); host pre-scales
                        # the Q bias by 1/8 so (raw+b)/8 == raw*0.125 + b/8
                        nc.scalar.activation(
                            dst, pss2[qc], IDENT,
                            bias=bqk_sb[:, ft : ft + 1],
                            scale=0.125 if ft < 6 else 1.0,
                        )
                for tt in ([] if 'qkv' in SKIP else range(8)):
                    v_tt(tt)

            # ---------------- stage B: attention ----------------
            with tc.tile_pool(name="late", bufs=1) as late:
              WpT_sb = late.tile([128, KT, D], F32R)
              nc.sync.dma_start(
                  out=WpT_sb, in_=WpT.rearrange("(k p) f -> p k f", p=128).bitcast(F32R)
              )
              with (
                tc.tile_pool(name="qkbuf", bufs=4) as qkbuf,
                tc.tile_pool(name="ysb", bufs=2) as ysb,
                tc.tile_pool(name="onebuf", bufs=1) as onebuf,
                tc.tile_pool(name="atp", bufs=2) as atp,
                tc.tile_pool(name="stat", bufs=2) as statp,
                tc.tile_pool(name="aps1", bufs=1, space="PSUM") as aps1,
                tc.tile_pool(name="apss", bufs=2, space="PSUM") as apss,
                tc.tile_pool(name="aps2", bufs=2, space="PSUM") as aps2,
                tc.tile_pool(name="apso", bufs=2, space="PSUM") as apso,
              ):
                def proj_tt(tt):
                    yt = ysb.tile([128, D], F32, tag="yt")
                    for ci, (f0, fl) in enumerate(((0, 512), (512, 256))):
                        psy = aps2.tile([128, 512], F32, tag="ps2")
                        for kt in range(KT):
                            nc.tensor.matmul(
                                psy[:, 0:fl],
                                (attnT[:, kt, tt * 128 : (tt + 1) * 128]),
                                (WpT_sb[:, kt, f0 : f0 + fl]),
                                start=(kt == 0),
                                stop=(kt == KT - 1),
                            )
                        nc.vector.tensor_add(
                            yt[:, f0 : f0 + fl], psy[:, 0:fl], bpb[:, f0 : f0 + fl]
                        )
                        if ci == 1:
                            nc.sync.dma_start(
                                out=y[tt * 128 : (tt + 1) * 128, :], in_=yt
                            )

                def phase1(h):
                    fq, off = h // 2, (h % 2) * 64
                    fk = 6 + fq
                    # Ktilde = [K^T ; ones], Qtilde = [Q^T/8 ; -(c_q)]
                    Kt = qkbuf.tile([65, S], F32R, tag="Kt")
                    nc.gpsimd.tensor_copy(Kt[0:64, :], QKT[off : off + 64, fk, :])
                    nc.gpsimd.memset(
                        Kt[64:65, :].bitcast(mybir.dt.uint32), 0x3F800000
                    )
                    Qt = qkbuf.tile([65, S], F32R, tag="Qt")
                    nc.gpsimd.tensor_copy(Qt[0:64, :], QKT[off : off + 64, fq, :])

                    use_lse = h in LSE_HEADS
                    mcol = statp.tile([128, 8], F32R, tag="mcol")
                    for qt in range(8):
                        mq = statp.tile([128, 2], F32, tag="mq")
                        for kc in range(2):
                            pss = apss.tile([128, 512], F32, tag="pss")
                            nc.tensor.matmul(
                                pss,
                                QKT[off : off + 64, fq, qt * 128 : (qt + 1) * 128],
                                QKT[off : off + 64, fk, kc * 512 : (kc + 1) * 512],
                                start=True,
                                stop=True,
                            )
                            if use_lse:
                                # ACT-side stats: l0 = sum exp(s/16); c_q
                                # bound = 16*ln(l0) - 40 (safe: slack<=104,
                                # window [-40, +64] around rowmax)
                                esc = statp.tile([128, 512], F32, tag="esc")
                                nc.scalar.activation(
                                    esc, pss, EXP, scale=0.0625,
                                    accum_out=mq[:, kc : kc + 1],
                                )
                            else:
                                nc.vector.reduce_max(
                                    mq[:, kc : kc + 1], pss, axis=AX
                                )
                        nc.vector.tensor_tensor(
                            mcol[:, qt : qt + 1], mq[:, 0:1], mq[:, 1:2],
                            OP.add if use_lse else OP.max,
                        )
                    if use_lse:
                        # bit-hack log2: c = 16*ln2*(bits/2^23 - 127) - 40;
                        # Qt row = -c = -16*ln2/2^23 * bits + (16*127*ln2 + 40)
                        mbits = statp.tile([128, 8], F32R, tag="mbits")
                        nc.vector.tensor_copy(
                            mbits, mcol.bitcast(mybir.dt.int32)
                        )
                        mcol = mbits
                    # row-ify -c: PE transpose, scale/offset on evac, DMA flatten
                    psT = aps1.tile([8, 128], F32R, tag="psT")
                    nc.tensor.transpose(psT, mcol, identr)
                    mstage = statp.tile([8, 128], F32, tag="mstage")
                    if use_lse:
                        nc.vector.tensor_scalar(
                            out=mstage, in0=psT,
                            scalar1=-16.0 * 0.6931471805599453 / 8388608.0,
                            scalar2=16.0 * 127.0 * 0.6931471805599453 + 40.0,
                            op0=OP.mult, op1=OP.add,
                        )
                    else:
                        nc.vector.tensor_scalar(
                            out=mstage, in0=psT, scalar1=-1.0, scalar2=-MARGIN,
                            op0=OP.mult, op1=OP.add,
                        )
                    nc.sync.dma_start(
                        out=Qt[64:65, :].bitcast(F32), in_=mstage
                    )
                    return Kt, Qt

                def phase2(h, Kt, Qt):
                    fq, off = h // 2, (h % 2) * 64
                    # S'^T = Ktilde^T Qtilde (k-major, max pre-subtracted); exp;
                    # then out^T(+denominator row) = Vaug^T A'^T ; normalize
                    for qc in range(2):
                        AT = atp.tile([128, 8, 512], BF16, tag="AT")
                        for kt in range(8):
                            ps2 = aps2.tile([128, 512], F32, tag="ps2")
                            nc.tensor.matmul(
                                ps2,
                                (Kt[:, kt * 128 : (kt + 1) * 128]),
                                (Qt[:, qc * 512 : (qc + 1) * 512]),
                                start=True,
                                stop=True,
                            )
                            nc.scalar.activation(AT[:, kt, :], ps2, EXP)

                        pso = apso.tile([65, 512], F32, tag="pso")
                        for kt in range(8):
                            nc.tensor.matmul(
                                pso,
                                (Vaug[:, kt, h, :]),
                                (AT[:, kt, :]),
                                start=(kt == 0),
                                stop=(kt == 7),
                            )
                        rec = onebuf.tile([1, 512], F32R, tag="rec")
                        with nc.allow_low_precision(reason="softmax 1/l in fp32r"):
                            nc.vector.reciprocal(rec, pso[64:65, :])
                        psb = aps1.tile([64, 512], F32, tag="psb")
                        nc.tensor.matmul(
                            psb, (ones1), (rec), start=True, stop=True
                        )
                        rbc = onebuf.tile([64, 512], F32, tag="rbc")
                        nc.vector.tensor_copy(rbc, psb)
                        nc.vector.tensor_mul(
                            attnT[off : off + 64, fq, qc * 512 : (qc + 1) * 512],
                            pso[0:64, :],
                            rbc,
                        )

                heads = [] if 'attn' in SKIP else list(range(NH))
                PIPE = int(os.environ.get("K_PIPE", "2"))
                pend = []
                for h in heads:
                    if PIPE:
                        pend.append((h, *phase1(h)))
                        if len(pend) > PIPE:
                            phase2(*pend.pop(0))
                    else:
                        phase2(h, *phase1(h))
                for item in pend:
                    phase2(*item)


                for tt in ([] if 'proj' in SKIP else range(8)):
                    proj_tt(tt)

    nc.compile()
    return nc


_NC = None


def _get_nc():
    global _NC
    if _NC is None:
        _NC = build_program()
    return _NC


def prep_inputs(x, qkv_w, qkv_b, U1_qkv, U2_qkv, U3_qkv, G_qkv,
                proj_w, proj_b, U1_p, U2_p, U3_p, G_p, task_idx):
    t = int(task_idx)
    f = np.float32
    x = np.asarray(x, f)
    qkv_w = np.asarray(qkv_w, f)
    qkv_b = np.asarray(qkv_b, f)
    proj_w = np.asarray(proj_w, f)
    proj_b = np.asarray(proj_b, f)

    Gt = np.einsum("pqv,v->pq", np.asarray(G_qkv, f), np.asarray(U3_qkv, f)[t])
    Wqkv = qkv_w + np.asarray(U1_qkv, f) @ Gt @ np.asarray(U2_qkv, f).T
    Gtp = np.einsum("pqv,v->pq", np.asarray(G_p, f), np.asarray(U3_p, f)[t])
    Wp = proj_w + np.asarray(U1_p, f) @ Gtp @ np.asarray(U2_p, f).T

    WqkT = np.ascontiguousarray(Wqkv[: 2 * D].T)
    WvT = np.ascontiguousarray(Wqkv[2 * D :].T)
    WpT = np.ascontiguousarray(Wp.T)
    bqk = np.ascontiguousarray(qkv_b[: 2 * D].reshape(12, 128).T)
    bqk[:, 0:6] *= 0.125  # Q bias pre-scaled (ACT evac applies scale to psum only)
    bv = np.ascontiguousarray(qkv_b[2 * D :])
    bp = proj_b

    B = x.shape[0]
    xr = x.reshape(B, S, D)
    in_maps = [
        dict(
            xT=np.ascontiguousarray(xr[c].T),
            WqkT=WqkT, WvT=WvT, WpT=WpT, bqk=bqk, bv=bv, bp=bp,
        )
        for c in range(B)
    ]
    return in_maps


def run(in_maps, trace=False):
    nc = _get_nc()
    res = run_bass_kernel_spmd(nc, in_maps, list(range(N_CORES)), trace=trace)
    return res


def kernel(x, **kw):
    B, H, W, C = x.shape
    in_maps = prep_inputs(x, **kw)
    res = run(in_maps)
    out = np.stack([np.asarray(res.results[c]["y"]) for c in range(B)])
    return out.reshape(B, H, W, C).astype(np.float32)



# revision 43
# speedup vs baseline: 1.3158x; 1.0259x over previous
"""Trainium2 Bass kernel for ToRA-adapted windowed attention block.

Math (per batch image, S=1024 tokens, dim=768, 12 heads x 64):
  qkv  = x @ (Wqkv + U1 Gt U2^T)^T + b          Gt = G . U3[task]
  q,k,v split; attn = softmax(q k^T / 8) v ; out = attn-merge
  y    = out @ (Wp + U1p Gtp U2p^T)^T + bp

Strategy: data-parallel over B=8 — one image per NeuronCore, no
collectives. Device pipeline is feature-major:
  - host pre-computes effective weights (tiny low-rank update) and
    pre-transposes weights + x so the contract dim lands on SBUF
    partitions.
  - QKV^T computed feature-major for Q,K (gives Q^T/K^T tiles directly);
    V computed token-major and augmented with a ones-column so the
    A^T @ V_aug matmul also yields softmax denominators for free.
  - scores are computed twice on PE (cheap with f32r @ 1 cyc/row):
    once q-major subsampled (stride 4) for row-max stats, once k-major
    with an appended (-max - margin) row folded into the contraction, so
    exp() needs no per-column bias and A^T comes out k-major, ready to
    contract with V.
  - proj consumes attention output feature-major; softmax 1/l scaling is
    applied during the attention evacuation via a PE rank-1 broadcast.
All big matmuls run as float32r (FP22 multiply, FP32 accumulate).
"""

import os
import sys
import numpy as np

sys.path.insert(0, "/opt/trn_rl_repo")

import concourse.bass as bass
import concourse.tile as tile
from concourse import bacc, mybir
from concourse.bass_utils import run_bass_kernel_spmd
from concourse.masks import make_identity

F32 = mybir.dt.float32
F32R = mybir.dt.float32r
BF16 = mybir.dt.bfloat16
AX = mybir.AxisListType.X
OP = mybir.AluOpType
EXP = mybir.ActivationFunctionType.Exp
LN = mybir.ActivationFunctionType.Ln
IDENT = mybir.ActivationFunctionType.Identity

D = 768          # model dim
KT = 6           # contract tiles over D
S = 1024         # tokens per image
NH = 12
HD = 64
MARGIN = 1.0     # safety margin over the stats-pass row-max

N_CORES = 8


def build_program():
    import os as _os
    SKIP = set(_os.environ.get("K_SKIP", "").split(","))
    LSE_HEADS = {
        int(v) for v in _os.environ.get("K_LSE", "2,6,10").split(",") if v != ""
    }
    nc = bacc.Bacc(
        "TRN2",
        target_bir_lowering=False,
        debug=False,
        enable_asserts=True,
        num_devices=N_CORES,
    )
    xT = nc.dram_tensor("xT", [D, S], F32, kind="ExternalInput").ap()
    WqkT = nc.dram_tensor("WqkT", [D, 2 * D], F32, kind="ExternalInput").ap()
    WvT = nc.dram_tensor("WvT", [D, D], F32, kind="ExternalInput").ap()
    WpT = nc.dram_tensor("WpT", [D, D], BF16, kind="ExternalInput").ap()
    bqk = nc.dram_tensor("bqk", [128, 12], F32, kind="ExternalInput").ap()
    bv = nc.dram_tensor("bv", [D], F32, kind="ExternalInput").ap()
    bp = nc.dram_tensor("bp", [D], F32, kind="ExternalInput").ap()
    y = nc.dram_tensor("y", [S, D], F32, kind="ExternalOutput").ap()

    def bcast128(v):
        return bass.AP(tensor=v.tensor, offset=v.offset, ap=[[0, 128], [1, D]])

    with tile.TileContext(nc) as tc:
        with tc.tile_pool(name="persist", bufs=1) as persist:
            # long-lived tiles
            QKT = persist.tile([128, 12, S], F32R)      # Q^T, K^T feature-major
            Vaug = persist.tile([128, 8, NH, 65], BF16)  # V tok-major + ones col
            attnT = persist.tile([128, KT, S], BF16)      # attn out^T, normalized
            bqk_sb = persist.tile([128, 12], F32)
            bvb3 = persist.tile([128, NH, HD], F32)
            bpb = persist.tile([128, D], F32)
            ident = persist.tile([128, 128], F32)
            mcolall = persist.tile([128, NH, 8], F32R)
            identr = persist.tile([128, 128], F32R)
            ones1 = persist.tile([1, 64], F32R)

            nc.sync.dma_start(out=bqk_sb, in_=bqk)
            nc.sync.dma_start(out=bvb3, in_=bcast128(bv))
            nc.sync.dma_start(out=bpb, in_=bcast128(bp))
            make_identity(nc, ident)
            nc.vector.tensor_scalar(
                out=identr, in0=ident, scalar1=1.0, scalar2=None, op0=OP.mult
            )
            ONE_F32 = 0x3F800000
            nc.vector.memset(ones1.bitcast(mybir.dt.uint32), ONE_F32)
            nc.vector.memset(Vaug, 1.0)

            # ---------------- stage A: QKV ----------------
            xT_sb = persist.tile([128, KT, S], F32R)
            WvT_sb = persist.tile([128, KT, D], F32R)
            with (
                tc.tile_pool(name="qkvw", bufs=1) as qkvw,
                tc.tile_pool(name="qkps", bufs=3, space="PSUM") as qkps,
                tc.tile_pool(name="vps", bufs=2, space="PSUM") as vps,
            ):
                WqkT_sb = qkvw.tile([128, KT, 2 * D], F32R)
                xT_r = xT.rearrange("(k p) t -> p k t", p=128).bitcast(F32R)
                WqkT_r = WqkT.rearrange("(k p) f -> p k f", p=128).bitcast(F32R)
                WvT_r = WvT.rearrange("(k p) f -> p k f", p=128).bitcast(F32R)

                def dma_wqk(ft, klo, khi):
                    nc.sync.dma_start(
                        out=WqkT_sb[:, klo:khi, ft * 128 : (ft + 1) * 128],
                        in_=WqkT_r[:, klo:khi, ft * 128 : (ft + 1) * 128],
                    )

                FTORD = [0, 6, 1, 7, 2, 8, 3, 9, 4, 10, 5, 11]
                # first ft pair interleaved with x so QK matmuls start ~2us in
                dma_wqk(0, 0, 3)
                nc.sync.dma_start(out=xT_sb[:, 0, :], in_=xT_r[:, 0, :])
                nc.sync.dma_start(out=xT_sb[:, 1, :], in_=xT_r[:, 1, :])
                dma_wqk(0, 3, 6)
                dma_wqk(6, 0, 6)
                for kt in range(2, KT):
                    nc.sync.dma_start(out=xT_sb[:, kt, :], in_=xT_r[:, kt, :])
                for ft in FTORD[2:]:
                    dma_wqk(ft, 0, 6)
                for kt in range(KT):
                    nc.sync.dma_start(out=WvT_sb[:, kt, :], in_=WvT_r[:, kt, :])

                # V token-major: one strided add into bf16 Vaug (+bias)
                def v_tt(tt):
                    psv = vps.tile([128, NH, HD], F32, tag="psv")
                    for kt in range(KT):
                        nc.tensor.matmul(
                            psv[:, 0:8, :],
                            (xT_sb[:, kt, tt * 128 : (tt + 1) * 128]),
                            (WvT_sb[:, kt, 0:512]),
                            start=(kt == 0),
                            stop=(kt == KT - 1),
                        )
                        nc.tensor.matmul(
                            psv[:, 8:12, :],
                            (xT_sb[:, kt, tt * 128 : (tt + 1) * 128]),
                            (WvT_sb[:, kt, 512:768]),
                            start=(kt == 0),
                            stop=(kt == KT - 1),
                        )
                    nc.vector.tensor_tensor(
                        Vaug[:, tt, :, 0:64], psv, bvb3, OP.add
                    )

                # Q^T / K^T feature-major: out[feat, tok]
                for ft in ([] if 'qkv' in SKIP else [0, 6, 1, 7, 2, 8, 3, 9, 4, 10, 5, 11]):
                    ps_a = qkps.tile([128, 512], F32, tag="qkps")
                    ps_b = qkps.tile([128, 512], F32, tag="qkps")
                    pss2 = [ps_a, ps_b]
                    for kt in range(KT):
                        for qc in range(2):
                            nc.tensor.matmul(
                                pss2[qc],
                                (WqkT_sb[:, kt, ft * 128 : (ft + 1) * 128]),
                                (xT_sb[:, kt, qc * 512 : (qc + 1) * 512]),
                                start=(kt == 0),
                                stop=(kt == KT - 1),
                            )
                    for qc in range(2):
                        dst = QKT[:, ft, qc * 512 : (qc + 1) * 512]
                        # ACT evac: Identity(psum*scale + bias); host pre-scales
                        # the Q bias by 1/8 so (raw+b)/8 == raw*0.125 + b/8
                        nc.scalar.activation(
                            dst, pss2[qc], IDENT,
                            bias=bqk_sb[:, ft : ft + 1],
                            scale=0.125 if ft < 6 else 1.0,
                        )
                for tt in ([] if 'qkv' in SKIP else range(4)):
                    v_tt(tt)

            # ---------------- stage B: attention ----------------
            with tc.tile_pool(name="late", bufs=1) as late:
              WpT_sb = late.tile([128, KT, D], BF16)
              nc.sync.dma_start(
                  out=WpT_sb, in_=WpT.rearrange("(k p) f -> p k f", p=128)
              )
              with (
                tc.tile_pool(name="qkbuf", bufs=4) as qkbuf,
                tc.tile_pool(name="ysb", bufs=2) as ysb,
                tc.tile_pool(name="onebuf", bufs=1) as onebuf,
                tc.tile_pool(name="atp", bufs=2) as atp,
                tc.tile_pool(name="stat", bufs=2) as statp,
                tc.tile_pool(name="aps1", bufs=1, space="PSUM") as aps1,
                tc.tile_pool(name="apss", bufs=2, space="PSUM") as apss,
                tc.tile_pool(name="aps2", bufs=2, space="PSUM") as aps2,
                tc.tile_pool(name="apso", bufs=2, space="PSUM") as apso,
              ):
                def proj_tt(tt):
                    yt = ysb.tile([128, D], F32, tag="yt")
                    for ci, (f0, fl) in enumerate(((0, 512), (512, 256))):
                        psy = aps2.tile([128, 512], F32, tag="ps2")
                        for kt in range(KT):
                            nc.tensor.matmul(
                                psy[:, 0:fl],
                                (attnT[:, kt, tt * 128 : (tt + 1) * 128]),
                                (WpT_sb[:, kt, f0 : f0 + fl]),
                                start=(kt == 0),
                                stop=(kt == KT - 1),
                            )
                        nc.vector.tensor_add(
                            yt[:, f0 : f0 + fl], psy[:, 0:fl], bpb[:, f0 : f0 + fl]
                        )
                        if ci == 1:
                            nc.sync.dma_start(
                                out=y[tt * 128 : (tt + 1) * 128, :], in_=yt
                            )

                def phase1(h):
                    fq, off = h // 2, (h % 2) * 64
                    fk = 6 + fq
                    # Ktilde = [K^T ; ones], Qtilde = [Q^T/8 ; -(c_q)]
                    Kt = qkbuf.tile([65, S], F32R, tag="Kt")
                    nc.gpsimd.tensor_copy(Kt[0:64, :], QKT[off : off + 64, fk, :])
                    nc.gpsimd.memset(
                        Kt[64:65, :].bitcast(mybir.dt.uint32), 0x3F800000
                    )
                    Qt = qkbuf.tile([65, S], F32R, tag="Qt")
                    nc.gpsimd.tensor_copy(Qt[0:64, :], QKT[off : off + 64, fq, :])

                    use_lse = h in LSE_HEADS
                    mcol = mcolall[:, h, :]
                    for qt in range(8):
                        mq = statp.tile([128, 2], F32, tag="mq")
                        for kc in range(2):
                            pss = apss.tile([128, 512], F32, tag="pss")
                            nc.tensor.matmul(
                                pss,
                                QKT[off : off + 64, fq, qt * 128 : (qt + 1) * 128],
                                QKT[off : off + 64, fk, kc * 512 : (kc + 1) * 512],
                                start=True,
                                stop=True,
                            )
                            if use_lse:
                                # ACT-side stats: l0 = sum exp(s/16); c_q
                                # bound = 16*ln(l0) - 40 (safe: slack<=104,
                                # window [-40, +64] around rowmax)
                                esc = statp.tile([128, 512], F32, tag="esc")
                                nc.scalar.activation(
                                    esc, pss, EXP, scale=0.0625,
                                    accum_out=mq[:, kc : kc + 1],
                                )
                            else:
                                nc.vector.reduce_max(
                                    mq[:, kc : kc + 1], pss, axis=AX
                                )
                        nc.vector.tensor_tensor(
                            mcol[:, qt : qt + 1], mq[:, 0:1], mq[:, 1:2],
                            OP.add if use_lse else OP.max,
                        )
                    if use_lse:
                        # bit-hack log2: c = 16*ln2*(bits/2^23 - 127) - 40;
                        # Qt row = -c = -16*ln2/2^23 * bits + (16*127*ln2 + 40)
                        mbits = statp.tile([128, 8], F32R, tag="mbits")
                        nc.vector.tensor_copy(
                            mbits, mcol.bitcast(mybir.dt.int32)
                        )
                        mcol = mbits
                    # row-ify -c: PE transpose, scale/offset on evac, DMA flatten
                    psT = aps1.tile([8, 128], F32R, tag="psT")
                    nc.tensor.transpose(psT, mcol, identr)
                    mstage = statp.tile([8, 128], F32, tag="mstage")
                    if use_lse:
                        nc.vector.tensor_scalar(
                            out=mstage, in0=psT,
                            scalar1=-16.0 * 0.6931471805599453 / 8388608.0,
                            scalar2=16.0 * 127.0 * 0.6931471805599453 + 40.0,
                            op0=OP.mult, op1=OP.add,
                        )
                    else:
                        nc.vector.tensor_scalar(
                            out=mstage, in0=psT, scalar1=-1.0, scalar2=-MARGIN,
                            op0=OP.mult, op1=OP.add,
                        )
                    nc.sync.dma_start(
                        out=Qt[64:65, :].bitcast(F32), in_=mstage
                    )
                    return Kt, Qt

                def phase2(h, Kt, Qt):
                    fq, off = h // 2, (h % 2) * 64
                    last = h == NH - 1 and 'proj' not in SKIP
                    # S'^T = Ktilde^T Qtilde (k-major, max pre-subtracted); exp;
                    # then out^T(+denominator row) = Vaug^T A'^T ; normalize
                    for qc in range(2):
                        AT = atp.tile([128, 8, 512], BF16, tag="AT")
                        for kt in range(8):
                            ps2 = aps2.tile([128, 512], F32, tag="ps2")
                            nc.tensor.matmul(
                                ps2,
                                (Kt[:, kt * 128 : (kt + 1) * 128]),
                                (Qt[:, qc * 512 : (qc + 1) * 512]),
                                start=True,
                                stop=True,
                            )
                            nc.scalar.activation(AT[:, kt, :], ps2, EXP)

                        pso = apso.tile([65, 512], F32, tag="pso")
                        for kt in range(8):
                            nc.tensor.matmul(
                                pso,
                                (Vaug[:, kt, h, :]),
                                (AT[:, kt, :]),
                                start=(kt == 0),
                                stop=(kt == 7),
                            )
                        rec = onebuf.tile([1, 512], F32R, tag="rec")
                        with nc.allow_low_precision(reason="softmax 1/l in fp32r"):
                            nc.vector.reciprocal(rec, pso[64:65, :])
                        psb = aps1.tile([64, 512], F32, tag="psb")
                        nc.tensor.matmul(
                            psb, (ones1), (rec), start=True, stop=True
                        )
                        rbc = onebuf.tile([64, 512], F32, tag="rbc")
                        nc.vector.tensor_copy(rbc, psb)
                        nc.vector.tensor_mul(
                            attnT[off : off + 64, fq, qc * 512 : (qc + 1) * 512],
                            pso[0:64, :],
                            rbc,
                        )

                def v_tt_b(tt):
                    # deferred V tile: PE-dense work under the early heads'
                    # DVE-paced stats; borrows apss score-psum slots
                    psva = apss.tile([128, 8, HD], F32, tag="pss", name="psva")
                    for kt in range(KT):
                        nc.tensor.matmul(
                            psva,
                            (xT_sb[:, kt, tt * 128 : (tt + 1) * 128]),
                            (WvT_sb[:, kt, 0:512]),
                            start=(kt == 0),
                            stop=(kt == KT - 1),
                        )
                    nc.vector.tensor_tensor(
                        Vaug[:, tt, 0:8, 0:64], psva, bvb3[:, 0:8, :], OP.add
                    )
                    psvb = apss.tile([128, 4, HD], F32, tag="pss", name="psvb")
                    for kt in range(KT):
                        nc.tensor.matmul(
                            psvb,
                            (xT_sb[:, kt, tt * 128 : (tt + 1) * 128]),
                            (WvT_sb[:, kt, 512:768]),
                            start=(kt == 0),
                            stop=(kt == KT - 1),
                        )
                    nc.vector.tensor_tensor(
                        Vaug[:, tt, 8:12, 0:64], psvb, bvb3[:, 8:12, :], OP.add
                    )

                heads = [] if 'attn' in SKIP else list(range(NH))
                PIPE = int(os.environ.get("K_PIPE", "2"))
                pend = []
                for h in heads:
                    if PIPE:
                        pend.append((h, *phase1(h)))
                        if h == 0 and 'qkv' not in SKIP:
                            v_tt_b(4)
                            v_tt_b(5)
                        if h == 1 and 'qkv' not in SKIP:
                            v_tt_b(6)
                            v_tt_b(7)
                        if len(pend) > PIPE:
                            phase2(*pend.pop(0))
                    else:
                        phase2(h, *phase1(h))
                for item in pend:
                    phase2(*item)


                for tt in ([] if 'proj' in SKIP else range(8)):
                    proj_tt(tt)

    nc.compile()
    return nc


_NC = None


def _get_nc():
    global _NC
    if _NC is None:
        _NC = build_program()
    return _NC


def prep_inputs(x, qkv_w, qkv_b, U1_qkv, U2_qkv, U3_qkv, G_qkv,
                proj_w, proj_b, U1_p, U2_p, U3_p, G_p, task_idx):
    t = int(task_idx)
    f = np.float32
    x = np.asarray(x, f)
    qkv_w = np.asarray(qkv_w, f)
    qkv_b = np.asarray(qkv_b, f)
    proj_w = np.asarray(proj_w, f)
    proj_b = np.asarray(proj_b, f)

    Gt = np.einsum("pqv,v->pq", np.asarray(G_qkv, f), np.asarray(U3_qkv, f)[t])
    Wqkv = qkv_w + np.asarray(U1_qkv, f) @ Gt @ np.asarray(U2_qkv, f).T
    Gtp = np.einsum("pqv,v->pq", np.asarray(G_p, f), np.asarray(U3_p, f)[t])
    Wp = proj_w + np.asarray(U1_p, f) @ Gtp @ np.asarray(U2_p, f).T

    WqkT = np.ascontiguousarray(Wqkv[: 2 * D].T)
    WvT = np.ascontiguousarray(Wqkv[2 * D :].T)
    import ml_dtypes
    WpT = np.ascontiguousarray(Wp.T.astype(ml_dtypes.bfloat16))
    bqk = np.ascontiguousarray(qkv_b[: 2 * D].reshape(12, 128).T)
    bqk[:, 0:6] *= 0.125  # Q bias pre-scaled (ACT evac applies scale to psum only)
    bv = np.ascontiguousarray(qkv_b[2 * D :])
    bp = proj_b

    B = x.shape[0]
    xr = x.reshape(B, S, D)
    in_maps = [
        dict(
            xT=np.ascontiguousarray(xr[c].T),
            WqkT=WqkT, WvT=WvT, WpT=WpT, bqk=bqk, bv=bv, bp=bp,
        )
        for c in range(B)
    ]
    return in_maps


def run(in_maps, trace=False):
    nc = _get_nc()
    res = run_bass_kernel_spmd(nc, in_maps, list(range(N_CORES)), trace=trace)
    return res


def kernel(x, **kw):
    B, H, W, C = x.shape
    in_maps = prep_inputs(x, **kw)
    res = run(in_maps)
    out = np.stack([np.asarray(res.results[c]["y"]) for c in range(B)])
    return out.reshape(B, H, W, C).astype(np.float32)

